# revision 15
# baseline (speedup 1.0000x reference)
"""Trainium2 Bass kernel for nn_Branch_2_36386962932308.

Network (per batch, feature-major planes [channels, L=h*w=4096]):
  stage1: Mamba(d=128, di=128, n=2, r=8, conv4) -> LN
  linear: 128->256 + SiLU   (stage-1 LN affine folded into the linear weight)
  stage2: Mamba(d=256, di=256, n=2, r=16, conv4) -> LN (affine applied on device)

Sharding: data-parallel over batch, one batch element per NeuronCore (8 cores).

Key restructurings:
  - Input x[b] is already the feature-major plane [c, h*w]; output likewise.
    No global transposes.
  - The causal depthwise conv (4 taps) is folded into in_proj: 4 shifted
    matmuls with host-precomputed weights (cw_k * win_x) accumulated in PSUM.
    3 leading zero columns on the input plane provide causal padding; a tiny
    correction fixes the folded input-bias on the first 3 columns.
  - The SSM scan uses the native DVE first-order recurrence
    tensor_tensor_scan (state = dA*state + dBu along the free dim), one
    instruction per [128, CH] chunk, chained across chunks via
    initial=prev[:, -1:].
  - B and C rows (per-timestep, shared across channels) are replicated across
    partitions with K=1 ones-matmuls on the PE, output read from PSUM.
  - out_proj emits time-major [T=128, dout] tiles so LayerNorm reduces along
    the free dim: ACT copy/square with accum_out gives per-timestep sum/sumsq;
    DVE tensor_scalar applies (x-mu)*rstd.
  - Stage-1 LN output returns to feature-major via PE transpose (fp32) or
    DMA-transpose (bf16, faster but ~3e-3 output error) selected by
    KBENCH_BF16=1; stage-2 output transposes on the PE, applies the LN affine
    per-channel, then DMAs to the output.

Self-contained: hardcodes all shapes; needs only concourse + numpy at runtime.
"""

import os
from contextlib import ExitStack

import numpy as np

import concourse.bass as bass
import concourse.bacc as bacc
import concourse.mybir as mybir
import concourse.tile as tile
from concourse.bass_utils import run_bass_kernel_spmd
from concourse.masks import make_identity

F32 = mybir.dt.float32
BF16 = mybir.dt.bfloat16
AF = mybir.ActivationFunctionType
ALU = mybir.AluOpType

NCORES = 8
LN_EPS = 1e-5
CH = 512          # main pipeline column chunk
SUB = 128         # out_proj / LN subchunk (time-major tile height)

last_exec_time_ns = None





# ----------------------------------------------------------------------------
# host-side weight preparation
# ----------------------------------------------------------------------------

def _prep_stage(p, d, di, r):
    win = np.asarray(p['win'], np.float32)
    b_in = np.asarray(p['bin'], np.float32)
    cw = np.asarray(p['cw'], np.float32)        # [di, 1, 4]
    cb = np.asarray(p['cb'], np.float32)
    wx = np.asarray(p['wx'], np.float32)        # [r+4, di]
    wdt = np.asarray(p['wdt'], np.float32)      # [di, r]
    bdt = np.asarray(p['bdt'], np.float32)
    alog = np.asarray(p['alog'], np.float32)    # [di, 2]
    dd = np.asarray(p['dd'], np.float32)
    wout = np.asarray(p['wout'], np.float32)    # [dout, di]

    winx, winz = win[:di], win[di:]
    w_k = np.stack([np.ascontiguousarray((cw[:, 0, k:k + 1] * winx).T)
                    for k in range(4)])          # [4, d, di]
    wz = np.ascontiguousarray(winz.T)            # [d, di]
    wxT = np.ascontiguousarray(wx.T)             # [di, r+4]
    wdtT = np.ascontiguousarray(wdt.T)           # [r, di]
    woutT = np.ascontiguousarray(wout.T)         # [di, dout]

    S = cw[:, 0, :].sum(1)
    silu_bias = cb + S * b_in[:di]
    bz = b_in[di:]
    # softplus is computed as -ln(sigmoid(-x)) (no Softplus LUT in this
    # compiler); the device keeps t = -dt, so store -bdt and -A = exp(alog).
    An = np.exp(alog)
    corr = np.stack([-(cw[:, 0, :3 - t].sum(1)) * b_in[:di] for t in range(3)], 1)
    cols = [silu_bias, bz, -bdt, An[:, 0], An[:, 1], dd,
            corr[:, 0], corr[:, 1], corr[:, 2]]
    sel = np.zeros((4, r + 4, 128), np.float32)
    for j in range(4):
        sel[j, r + j, :] = 1.0
    return w_k, wz, wxT, wdtT, woutT, np.stack(cols, 1).astype(np.float32), sel


def prep_weights(inputs, use_bf16):
    s1 = {k[3:]: inputs[k] for k in inputs if k.startswith('s1_')}
    s2 = {k[3:]: inputs[k] for k in inputs if k.startswith('s2_')}
    w1k, w1z, wx1, wdt1, wout1, cols1, sel1 = _prep_stage(s1, 128, 128, 8)
    w2k, w2z, wx2, wdt2, wout2, cols2, sel2 = _prep_stage(s2, 256, 256, 16)
    lnw2 = np.asarray(s2['lnw'], np.float32)
    lnb2 = np.asarray(s2['lnb'], np.float32)
    cols2 = np.concatenate([cols2, lnw2[:, None], lnb2[:, None]], 1)
    cols2 = np.ascontiguousarray(cols2, dtype=np.float32)

    lin_w = np.asarray(inputs['lin_w'], np.float32)
    lin_b = np.asarray(inputs['lin_b'], np.float32)
    lnw1 = np.asarray(s1['lnw'], np.float32)
    lnb1 = np.asarray(s1['lnb'], np.float32)
    linw = np.ascontiguousarray((lin_w * lnw1[None, :]).T)   # [128, 256]
    if use_bf16:
        linw = linw.astype(mybir.dt.np(BF16))
    linb = (lin_w @ lnb1 + lin_b).astype(np.float32)[:, None]

    return {
        'w1k': w1k, 'w1z': w1z, 'wx1': wx1, 'wdt1': wdt1, 'wout1': wout1,
        'cols1': cols1,
        'w2k': w2k, 'w2z': w2z, 'wx2': wx2, 'wdt2': wdt2, 'wout2': wout2,
        'cols2': cols2, 'sel1': sel1, 'sel2': sel2,
        'linw': linw, 'linb': linb,
    }


# ----------------------------------------------------------------------------
# device program
# ----------------------------------------------------------------------------

def _tile(pool, shape, dtype, tag, bufs=None):
    return pool.tile(shape, dtype, tag=tag, name=tag, bufs=bufs)


def _ssm_stage(nc, pools, cfg):
    """Emit one Mamba stage over the whole sequence. See build_program."""
    sb = pools['sb']
    ps_mm, ps_rep, ps_o = pools['mm'], pools['rep'], pools['o']
    L, P_in, P, r, dout = cfg['L'], cfg['P_in'], cfg['P'], cfg['r'], cfg['dout']
    rw = r + 4
    planes = cfg['in_planes']
    wk, wz, wxs, wdts, wouts, cols = (cfg['wk_sb'], cfg['wz_sb'], cfg['wx_sb'],
                                      cfg['wdt_sb'], cfg['wout_sb'],
                                      cfg['cols_sb'])

    hs_prev = [[None] * P, [None] * P]
    for ci in range(L // CH):
        c0 = ci * CH
        # ---- in_proj (conv folded: 4 shifted taps) + z ----
        xc_sb, sz_sb, dt_sb = [], [], []
        dA_sb = [[None] * P, [None] * P]
        for mi in range(P):
            ms = slice(mi * 128, (mi + 1) * 128)
            xc_ps = _tile(ps_mm, [128, CH], F32, "mm", 2)
            nmm = 4 * P_in
            i = 0
            for k in range(4):
                for kt in range(P_in):
                    nc.tensor.matmul(
                        xc_ps[:], wk[k][kt][:, ms],
                        planes[kt][:, c0 + k: c0 + k + CH],
                        start=(i == 0), stop=(i == nmm - 1))
                    i += 1
            if ci == 0:
                nc.vector.tensor_add(xc_ps[:, 0:3], xc_ps[:, 0:3],
                                     cols[mi][:, 6:9])
            t_xc = _tile(sb, [128, CH], F32, "xc")
            nc.scalar.activation(t_xc[:], xc_ps[:], AF.Silu,
                                 bias=cols[mi][:, 0:1])
            xc_sb.append(t_xc)

            z_ps = _tile(ps_mm, [128, CH], F32, "mm", 2)
            for kt in range(P_in):
                nc.tensor.matmul(z_ps[:], wz[kt][:, ms],
                                 planes[kt][:, c0 + 3: c0 + 3 + CH],
                                 start=(kt == 0), stop=(kt == P_in - 1))
            t_sz = _tile(sb, [128, CH], F32, "sz")
            nc.scalar.activation(t_sz[:], z_ps[:], AF.Silu,
                                 bias=cols[mi][:, 1:2])
            sz_sb.append(t_sz)

        # ---- wx projection -> dtin rows + B/C rows ----
        xdbl_ps = _tile(ps_mm, [128, CH], F32, "mm", 2)
        for kt in range(P_in):
            nc.tensor.matmul(xdbl_ps[:rw, :], wxs[kt][:], xc_sb[kt][:],
                             start=(kt == 0), stop=(kt == P_in - 1))
        xdbl = _tile(sb, [rw, CH], F32, "xdbl")
        nc.scalar.activation(xdbl[:], xdbl_ps[:rw, :], AF.Identity)

        # ---- dt = softplus(wdt @ dtin + bdt); dA_n = exp(A_n * dt) ----
        for mi in range(P):
            ms = slice(mi * 128, (mi + 1) * 128)
            dt_ps = _tile(ps_mm, [128, CH], F32, "mm", 2)
            nc.tensor.matmul(dt_ps[:], wdts[:, ms], xdbl[:r, :])
            t_sg = _tile(sb, [128, CH], F32, "sg")
            nc.scalar.activation(t_sg[:], dt_ps[:], AF.Sigmoid,
                                 bias=cols[mi][:, 2:3], scale=-1.0)
            t_dt = _tile(sb, [128, CH], F32, "dt")
            nc.scalar.activation(t_dt[:], t_sg[:], AF.Ln)
            dt_sb.append(t_dt)
            for n in range(2):
                t_dA = _tile(sb, [128, CH], F32, f"dA{n}")
                nc.scalar.activation(t_dA[:], t_dt[:], AF.Exp,
                                     scale=cols[mi][:, 3 + n: 4 + n])
                dA_sb[n][mi] = t_dA

        # ---- replicate B/C rows across partitions (K=1 PE matmuls) ----
        repB = _tile(ps_rep, [128, 2 * CH], F32, "repB", 1)
        for j in range(2):
            nc.tensor.matmul(repB[:, j * CH:(j + 1) * CH], cfg['sel_sb'][j][:],
                             xdbl[:])

        # ---- scan inputs + scan ----
        hs = [[None] * P, [None] * P]
        u_sb = []
        for mi in range(P):
            t_u = _tile(sb, [128, CH], F32, "u")
            nc.vector.tensor_mul(t_u[:], dt_sb[mi][:], xc_sb[mi][:])
            u_sb.append(t_u)
            for n in range(2):
                t_dbu = _tile(sb, [128, CH], F32, f"dbu{n}")
                nc.vector.tensor_mul(t_dbu[:], t_u[:],
                                     repB[:, n * CH:(n + 1) * CH])
                t_hs = _tile(sb, [128, CH], F32, f"hs{n}", 3)
                init = 0.0 if ci == 0 else hs_prev[n][mi][:, CH - 1:CH]
                nc.vector.tensor_tensor_scan(t_hs[:], dA_sb[n][mi][:],
                                             t_dbu[:], init, ALU.mult, ALU.add)
                hs[n][mi] = t_hs
                hs_prev[n][mi] = t_hs

        repC = _tile(ps_rep, [128, 2 * CH], F32, "repC", 1)
        for j in range(2):
            nc.tensor.matmul(repC[:, j * CH:(j + 1) * CH],
                             cfg['sel_sb'][2 + j][:], xdbl[:])

        # ---- y = (hs0*C0 + hs1*C1 + dd*xc) * silu(z) ----
        yg_sb = []
        for mi in range(P):
            t_m0 = _tile(sb, [128, CH], F32, "m0")
            nc.vector.tensor_mul(t_m0[:], hs[0][mi][:], repC[:, 0:CH])
            t_y = _tile(sb, [128, CH], F32, "y")
            nc.vector.tensor_mul(t_y[:], hs[1][mi][:], repC[:, CH:2 * CH])
            nc.vector.tensor_add(t_y[:], t_y[:], t_m0[:])
            nc.vector.scalar_tensor_tensor(t_y[:], xc_sb[mi][:],
                                           cols[mi][:, 5:6], t_y[:],
                                           ALU.mult, ALU.subtract)
            t_yg = _tile(sb, [128, CH], F32, "yg")
            nc.vector.tensor_mul(t_yg[:], t_y[:], sz_sb[mi][:])
            yg_sb.append(t_yg)

        # ---- out_proj (time-major) + LayerNorm ----
        nsub = CH // SUB
        ssum = _tile(sb, [SUB, nsub], F32, "ssum")
        ssq = _tile(sb, [SUB, nsub], F32, "ssq")
        yp_tiles = []
        for g in range(nsub):
            cs = slice(g * SUB, (g + 1) * SUB)
            yp_ps = _tile(ps_o, [SUB, dout], F32, "yp", 1)
            for mi in range(P):
                nc.tensor.matmul(yp_ps[:], yg_sb[mi][:, cs], wouts[mi][:],
                                 start=(mi == 0), stop=(mi == P - 1))
            yp = _tile(sb, [SUB, dout], F32, "ypsb", 4)
            nc.scalar.activation(yp[:], yp_ps[:], AF.Identity,
                                 accum_out=ssum[:, g:g + 1])
            scr = _tile(sb, [SUB, dout], F32, "scr")
            nc.scalar.activation(scr[:], yp_ps[:], AF.Square,
                                 accum_out=ssq[:, g:g + 1])
            yp_tiles.append(yp)
        mu = _tile(sb, [SUB, nsub], F32, "mu")
        nc.vector.tensor_scalar(mu[:], ssum[:], 1.0 / dout, None,
                                ALU.mult, ALU.bypass)
        musq = _tile(sb, [SUB, nsub], F32, "musq")
        nc.vector.tensor_mul(musq[:], mu[:], mu[:])
        var = _tile(sb, [SUB, nsub], F32, "var")
        nc.vector.scalar_tensor_tensor(var[:], ssq[:], 1.0 / dout, musq[:],
                                       ALU.mult, ALU.subtract)
        nc.vector.tensor_scalar(var[:], var[:], LN_EPS, None,
                                ALU.add, ALU.bypass)
        std = _tile(sb, [SUB, nsub], F32, "std")
        nc.scalar.activation(std[:], var[:], AF.Sqrt)
        rstd = _tile(sb, [SUB, nsub], F32, "rstd")
        nc.vector.reciprocal(rstd[:], std[:])
        for g in range(nsub):
            tn = _tile(sb, [SUB, dout], cfg['tn_dtype'], "tn")
            nc.vector.tensor_scalar(tn[:], yp_tiles[g][:], mu[:, g:g + 1],
                                    rstd[:, g:g + 1], ALU.subtract, ALU.mult)
            cfg['emit'](tn, c0, g)


def build_program(L=4096, use_bf16=False):
    nc = bacc.Bacc()
    dp = nc.declare_dram_parameter
    x_d = dp("x", [128, L], F32, isOutput=False)
    w1k_d = dp("w1k", [4, 128, 128], F32, isOutput=False)
    w1z_d = dp("w1z", [128, 128], F32, isOutput=False)
    wx1_d = dp("wx1", [128, 12], F32, isOutput=False)
    wdt1_d = dp("wdt1", [8, 128], F32, isOutput=False)
    wout1_d = dp("wout1", [128, 128], F32, isOutput=False)
    cols1_d = dp("cols1", [128, 9], F32, isOutput=False)
    w2k_d = dp("w2k", [4, 256, 256], F32, isOutput=False)
    w2z_d = dp("w2z", [256, 256], F32, isOutput=False)
    wx2_d = dp("wx2", [256, 20], F32, isOutput=False)
    wdt2_d = dp("wdt2", [16, 256], F32, isOutput=False)
    wout2_d = dp("wout2", [256, 256], F32, isOutput=False)
    cols2_d = dp("cols2", [256, 11], F32, isOutput=False)
    sel1_d = dp("sel1", [4, 12, 128], F32, isOutput=False)
    sel2_d = dp("sel2", [4, 20, 128], F32, isOutput=False)
    linw_d = dp("linw", [128, 256], BF16 if use_bf16 else F32, isOutput=False)
    linb_d = dp("linb", [256, 1], F32, isOutput=False)
    out_d = dp("out", [256, L], F32, isOutput=True)

    dma = nc.sync.dma_start
    t1_dtype = BF16 if use_bf16 else F32

    with tile.TileContext(nc) as tc, ExitStack() as ctx:
        consts = ctx.enter_context(tc.tile_pool(name="consts", bufs=1))
        planes = ctx.enter_context(tc.tile_pool(name="planes", bufs=1))
        sb = ctx.enter_context(tc.tile_pool(name="sb", bufs=2))
        ps_mm = ctx.enter_context(
            tc.tile_pool(name="psmm", bufs=2, space=bass.MemorySpace.PSUM))
        ps_rep = ctx.enter_context(
            tc.tile_pool(name="psrep", bufs=1, space=bass.MemorySpace.PSUM))
        ps_o = ctx.enter_context(
            tc.tile_pool(name="pso", bufs=1, space=bass.MemorySpace.PSUM))
        pools = {'sb': sb, 'mm': ps_mm, 'rep': ps_rep, 'o': ps_o}

        _ld = [0]

        def load(dram_ap, shape, dtype=F32):
            _ld[0] += 1
            t = consts.tile(shape, dtype, tag=f"w{_ld[0]}", name=f"w{_ld[0]}")
            dma(t[:], dram_ap)
            return t

        w1k_sb = [[load(w1k_d[k], [128, 128])] for k in range(4)]
        w1z_sb = [load(w1z_d[:], [128, 128])]
        wx1_sb = [load(wx1_d[:], [128, 12])]
        wdt1_sb = load(wdt1_d[:], [8, 128])
        wout1_sb = [load(wout1_d[:], [128, 128])]
        cols1_sb = [load(cols1_d[:], [128, 9])]
        w2k_sb = [[load(w2k_d[k, kt * 128:(kt + 1) * 128], [128, 256])
                   for kt in range(2)] for k in range(4)]
        w2z_sb = [load(w2z_d[kt * 128:(kt + 1) * 128], [128, 256])
                  for kt in range(2)]
        wx2_sb = [load(wx2_d[kt * 128:(kt + 1) * 128], [128, 20])
                  for kt in range(2)]
        wdt2_sb = load(wdt2_d[:], [16, 256])
        wout2_sb = [load(wout2_d[kt * 128:(kt + 1) * 128], [128, 256])
                    for kt in range(2)]
        cols2_sb = [load(cols2_d[kt * 128:(kt + 1) * 128], [128, 11])
                    for kt in range(2)]
        sel1_sb = [load(sel1_d[j], [12, 128]) for j in range(4)]
        sel2_sb = [load(sel2_d[j], [20, 128]) for j in range(4)]
        linw_sb = load(linw_d[:], [128, 256], BF16 if use_bf16 else F32)
        linb_sb = [load(linb_d[kt * 128:(kt + 1) * 128], [128, 1])
                   for kt in range(2)]

        ident = consts.tile([128, 128], F32, tag="ident", name="ident")
        make_identity(nc, ident)

        xpad = planes.tile([128, L + 3], F32, tag="xpad", name="xpad")
        nc.gpsimd.memset(xpad[:, 0:3], 0.0)
        dma(xpad[:, 3:], x_d[:])
        t1n = planes.tile([128, L], t1_dtype, tag="t1n", name="t1n")
        t2pad = [planes.tile([128, L + 3], F32, tag=f"t2pad{mi}",
                               name=f"t2pad{mi}") for mi in range(2)]
        for mi in range(2):
            nc.gpsimd.memset(t2pad[mi][:, 0:3], 0.0)

        # ---- stage 1 ----
        if use_bf16:
            def emit1(tn, c0, g):
                nc.sync.dma_start_transpose(
                    t1n[:, c0 + g * SUB: c0 + (g + 1) * SUB], tn[:])
        else:
            def emit1(tn, c0, g):
                tf = _tile(ps_o, [128, SUB], F32, "tf", 1)
                nc.tensor.transpose(tf[:], tn[:], ident[:])
                nc.scalar.activation(
                    t1n[:, c0 + g * SUB: c0 + (g + 1) * SUB], tf[:],
                    AF.Identity)

        _ssm_stage(nc, pools, dict(
            L=L, P_in=1, P=1, r=8, dout=128, in_planes=[xpad],
            wk_sb=w1k_sb, wz_sb=w1z_sb, wx_sb=wx1_sb, wdt_sb=wdt1_sb,
            wout_sb=wout1_sb, cols_sb=cols1_sb, sel_sb=sel1_sb,
            tn_dtype=t1_dtype, emit=emit1))

        # ---- linear + silu ----
        for c0 in range(0, L, CH):
            for mi in range(2):
                ms = slice(mi * 128, (mi + 1) * 128)
                lp = _tile(ps_mm, [128, CH], F32, "mm", 2)
                nc.tensor.matmul(lp[:], linw_sb[:, ms], t1n[:, c0:c0 + CH])
                nc.scalar.activation(t2pad[mi][:, 3 + c0: 3 + c0 + CH], lp[:],
                                     AF.Silu, bias=linb_sb[mi][:, 0:1])

        # ---- stage 2 ----
        def emit2(tn, c0, g):
            for ct in range(2):
                tf = _tile(ps_o, [128, SUB], F32, "tf", 1)
                nc.tensor.transpose(tf[:], tn[:, ct * 128:(ct + 1) * 128],
                                    ident[:])
                of = _tile(sb, [128, SUB], F32, "of")
                nc.vector.tensor_scalar(of[:], tf[:], cols2_sb[ct][:, 9:10],
                                        cols2_sb[ct][:, 10:11],
                                        ALU.mult, ALU.add)
                dma(out_d[ct * 128:(ct + 1) * 128,
                          c0 + g * SUB: c0 + (g + 1) * SUB], of[:])

        _ssm_stage(nc, pools, dict(
            L=L, P_in=2, P=2, r=16, dout=256, in_planes=t2pad,
            wk_sb=w2k_sb, wz_sb=w2z_sb, wx_sb=wx2_sb, wdt_sb=wdt2_sb,
            wout_sb=wout2_sb, cols_sb=cols2_sb, sel_sb=sel2_sb,
            tn_dtype=F32, emit=emit2))

    nc.finalize()
    return nc


# ----------------------------------------------------------------------------
# entry point
# ----------------------------------------------------------------------------

_NC = {}


def kernel(**inputs):
    global last_exec_time_ns
    use_bf16 = os.environ.get("KBENCH_BF16", "0") == "1"
    inputs = {k: np.asarray(v) for k, v in inputs.items()}
    weights = prep_weights(inputs, use_bf16)
    x = inputs['x'].astype(np.float32)          # [8, 128, 64, 64]
    b, c, h, w = x.shape
    L = h * w

    key = (L, use_bf16)
    if key not in _NC:
        _NC[key] = build_program(L, use_bf16)

    in_maps = [dict(weights, x=np.ascontiguousarray(x[i].reshape(c, L)))
               for i in range(NCORES)]
    res = run_bass_kernel_spmd(
        _NC[key], in_maps, list(range(NCORES)),
        trace=bool(os.environ.get("KBENCH_TRACE")))
    last_exec_time_ns = res.exec_time_ns
    out = np.stack([np.asarray(res.results[i]['out'], np.float32)
                    .reshape(256, h, w) for i in range(NCORES)])
    return out


# revision 16
# speedup vs baseline: 1.0566x; 1.0566x over previous
"""Trainium2 Bass kernel for nn_Branch_2_36386962932308.

Network (per batch, feature-major planes [channels, L=h*w=4096]):
  stage1: Mamba(d=128, di=128, n=2, r=8, conv4) -> LN
  linear: 128->256 + SiLU   (stage-1 LN affine folded into the linear weight)
  stage2: Mamba(d=256, di=256, n=2, r=16, conv4) -> LN (affine applied on device)

Sharding: data-parallel over batch, one batch element per NeuronCore (8 cores).

Key restructurings:
  - Input x[b] is already the feature-major plane [c, h*w]; output likewise.
    No global transposes anywhere.
  - The causal depthwise conv (4 taps) is folded into in_proj: 4 shifted
    matmuls with host-precomputed weights (cw_k * win_x) accumulated in PSUM.
    3 leading zero columns on the input plane provide causal padding; a tiny
    correction fixes the folded input-bias on the first 3 columns.
  - The SSM scan uses the native DVE first-order recurrence
    tensor_tensor_scan (state = dA*state + dBu along the free dim), one
    instruction per [128, CH] chunk, chained across chunks via
    initial=prev[:, -1:].
  - B and C rows (per-timestep, shared across channels) are replicated across
    partitions with selection-matrix matmuls on the PE (rep = sel_j.T @ xdbl).
  - out_proj emits time-major [T=128, dout] tiles so LayerNorm reduces along
    the free dim: ACT copy/square with accum_out gives per-timestep sum/sumsq;
    DVE tensor_scalar applies (x-mu)*rstd.
  - ACT table-set discipline: each span runs a SiLU phase (silu_and_others
    set), then everything else uses only natural_log_exp_and_others —
    softplus(x) = Ln(Exp(x) + 1), rstd = Exp(-0.5*Ln(var+eps)); Identity and
    Square are fillers present in every set. This keeps table swaps to a few
    per kernel instead of several per chunk (~2.7us each).
  - Stage-1 LN output returns to feature-major via PE transpose (fp32) or
    DMA-transpose (bf16, KBENCH_BF16=1); stage-2 output transposes on the PE,
    applies the LN affine per-channel, then DMAs out.

Self-contained: hardcodes all shapes; needs only concourse + numpy at runtime.
"""

import os
from contextlib import ExitStack

import numpy as np

import concourse.bass as bass
import concourse.bacc as bacc
import concourse.mybir as mybir
import concourse.tile as tile
from concourse.bass_utils import run_bass_kernel_spmd
from concourse.masks import make_identity

F32 = mybir.dt.float32
BF16 = mybir.dt.bfloat16
AF = mybir.ActivationFunctionType
ALU = mybir.AluOpType

NCORES = 8
LN_EPS = 1e-5
CH = 512          # pipeline column chunk (one PSUM bank at fp32)
SUB = 128         # out_proj / LN subchunk (time-major tile height)
SPAN = 2048       # ACT table-set phase width

last_exec_time_ns = None


# ----------------------------------------------------------------------------
# host-side weight preparation
# ----------------------------------------------------------------------------

def _prep_stage(p, d, di, r):
    win = np.asarray(p['win'], np.float32)
    b_in = np.asarray(p['bin'], np.float32)
    cw = np.asarray(p['cw'], np.float32)        # [di, 1, 4]
    cb = np.asarray(p['cb'], np.float32)
    wx = np.asarray(p['wx'], np.float32)        # [r+4, di]
    wdt = np.asarray(p['wdt'], np.float32)      # [di, r]
    bdt = np.asarray(p['bdt'], np.float32)
    alog = np.asarray(p['alog'], np.float32)    # [di, 2]
    dd = np.asarray(p['dd'], np.float32)
    wout = np.asarray(p['wout'], np.float32)    # [dout, di]

    winx, winz = win[:di], win[di:]
    w_k = np.stack([np.ascontiguousarray((cw[:, 0, k:k + 1] * winx).T)
                    for k in range(4)])          # [4, d, di]
    wz = np.ascontiguousarray(winz.T)            # [d, di]
    wxT = np.ascontiguousarray(wx.T)             # [di, r+4]
    wdtT = np.ascontiguousarray(wdt.T)           # [r, di]
    woutT = np.ascontiguousarray(wout.T)         # [di, dout]

    S = cw[:, 0, :].sum(1)
    silu_bias = cb + S * b_in[:di]
    bz = b_in[di:]
    A = -np.exp(alog)                            # [di, 2] (negative)
    corr = np.stack([-(cw[:, 0, :3 - t].sum(1)) * b_in[:di] for t in range(3)], 1)
    cols = [silu_bias, bz, bdt, A[:, 0], A[:, 1], dd,
            corr[:, 0], corr[:, 1], corr[:, 2]]
    sel = np.zeros((4, r + 4, 128), np.float32)
    for j in range(4):
        sel[j, r + j, :] = 1.0
    return w_k, wz, wxT, wdtT, woutT, np.stack(cols, 1).astype(np.float32), sel


def prep_weights(inputs, use_bf16):
    s1 = {k[3:]: inputs[k] for k in inputs if k.startswith('s1_')}
    s2 = {k[3:]: inputs[k] for k in inputs if k.startswith('s2_')}
    w1k, w1z, wx1, wdt1, wout1, cols1, sel1 = _prep_stage(s1, 128, 128, 8)
    w2k, w2z, wx2, wdt2, wout2, cols2, sel2 = _prep_stage(s2, 256, 256, 16)
    lnw2 = np.asarray(s2['lnw'], np.float32)
    lnb2 = np.asarray(s2['lnb'], np.float32)
    cols2 = np.concatenate([cols2, lnw2[:, None], lnb2[:, None]], 1)
    cols2 = np.ascontiguousarray(cols2, dtype=np.float32)

    lin_w = np.asarray(inputs['lin_w'], np.float32)
    lin_b = np.asarray(inputs['lin_b'], np.float32)
    lnw1 = np.asarray(s1['lnw'], np.float32)
    lnb1 = np.asarray(s1['lnb'], np.float32)
    linw = np.ascontiguousarray((lin_w * lnw1[None, :]).T)   # [128, 256]
    if use_bf16:
        linw = linw.astype(mybir.dt.np(BF16))
    linb = (lin_w @ lnb1 + lin_b).astype(np.float32)[:, None]

    return {
        'w1k': w1k, 'w1z': w1z, 'wx1': wx1, 'wdt1': wdt1, 'wout1': wout1,
        'cols1': cols1,
        'w2k': w2k, 'w2z': w2z, 'wx2': wx2, 'wdt2': wdt2, 'wout2': wout2,
        'cols2': cols2, 'sel1': sel1, 'sel2': sel2,
        'linw': linw, 'linb': linb,
    }


# ----------------------------------------------------------------------------
# device program
# ----------------------------------------------------------------------------

def _tile(pool, shape, dtype, tag, bufs=None):
    return pool.tile(shape, dtype, tag=tag, name=tag, bufs=bufs)


def _stage_phase_a(nc, pools, cfg, s0):
    """in_proj (conv-folded) + z + SiLU for one span -> xc/sz span planes."""
    ps_mm = pools['mm']
    P_in, P = cfg['P_in'], cfg['P']
    planes, wk, wz, cols = (cfg['in_planes'], cfg['wk_sb'], cfg['wz_sb'],
                            cfg['cols_sb'])
    xc_sp, sz_sp = cfg['xc_sp'], cfg['sz_sp']
    for ci in range(SPAN // CH):
        c0 = s0 + ci * CH
        lc = ci * CH
        for mi in range(P):
            ms = slice(mi * 128, (mi + 1) * 128)
            xc_ps = _tile(ps_mm, [128, CH], F32, "mm", 2)
            nmm = 4 * P_in
            i = 0
            for k in range(4):
                for kt in range(P_in):
                    nc.tensor.matmul(
                        xc_ps[:], wk[k][kt][:, ms],
                        planes[kt][:, c0 + k: c0 + k + CH],
                        start=(i == 0), stop=(i == nmm - 1))
                    i += 1
            if c0 == 0:
                nc.vector.tensor_add(xc_ps[:, 0:3], xc_ps[:, 0:3],
                                     cols[mi][:, 6:9])
            nc.scalar.activation(xc_sp[mi][:, lc:lc + CH], xc_ps[:], AF.Silu,
                                 bias=cols[mi][:, 0:1])

            z_ps = _tile(ps_mm, [128, CH], F32, "mm", 2)
            for kt in range(P_in):
                nc.tensor.matmul(z_ps[:], wz[kt][:, ms],
                                 planes[kt][:, c0 + 3: c0 + 3 + CH],
                                 start=(kt == 0), stop=(kt == P_in - 1))
            nc.scalar.activation(sz_sp[mi][:, lc:lc + CH], z_ps[:], AF.Silu,
                                 bias=cols[mi][:, 1:2])


def _stage_phase_b(nc, pools, cfg, s0, hs_prev):
    """Everything after SiLU for one span (natural_log_exp table set only)."""
    sb = pools['sb']
    ps_mm, ps_rep, ps_o = pools['mm'], pools['rep'], pools['o']
    P_in, P, r, dout = cfg['P_in'], cfg['P'], cfg['r'], cfg['dout']
    rw = r + 4
    wxs, wdts, wouts, cols = (cfg['wx_sb'], cfg['wdt_sb'], cfg['wout_sb'],
                              cfg['cols_sb'])
    xc_sp, sz_sp = cfg['xc_sp'], cfg['sz_sp']

    for ci in range(SPAN // CH):
        c0 = s0 + ci * CH
        lc = ci * CH
        lcs = slice(lc, lc + CH)
        # ---- wx projection -> dtin rows + B/C rows ----
        xdbl_ps = _tile(ps_mm, [128, CH], F32, "mm", 2)
        for kt in range(P_in):
            nc.tensor.matmul(xdbl_ps[:rw, :], wxs[kt][:], xc_sp[kt][:, lcs],
                             start=(kt == 0), stop=(kt == P_in - 1))
        xdbl = _tile(sb, [rw, CH], F32, "xdbl")
        nc.scalar.activation(xdbl[:], xdbl_ps[:rw, :], AF.Identity)

        # ---- dt = softplus = Ln(Exp(raw + bdt) + 1); dA_n = exp(A_n*dt) ----
        dt_sb = []
        dA_sb = [[None] * P, [None] * P]
        for mi in range(P):
            ms = slice(mi * 128, (mi + 1) * 128)
            dt_ps = _tile(ps_mm, [128, CH], F32, "mm", 2)
            nc.tensor.matmul(dt_ps[:], wdts[:, ms], xdbl[:r, :])
            t_e = _tile(sb, [128, CH], F32, "edt")
            nc.scalar.activation(t_e[:], dt_ps[:], AF.Exp,
                                 bias=cols[mi][:, 2:3])
            t_dt = _tile(sb, [128, CH], F32, "dt")
            nc.scalar.activation(t_dt[:], t_e[:], AF.Ln, bias=1.0)
            dt_sb.append(t_dt)
            for n in range(2):
                t_dA = _tile(sb, [128, CH], F32, f"dA{n}")
                nc.scalar.activation(t_dA[:], t_dt[:], AF.Exp,
                                     scale=cols[mi][:, 3 + n: 4 + n])
                dA_sb[n][mi] = t_dA

        # ---- replicate B rows; scan inputs + scan ----
        repB = _tile(ps_rep, [128, 2 * CH], F32, "repB", 1)
        for j in range(2):
            nc.tensor.matmul(repB[:, j * CH:(j + 1) * CH], cfg['sel_sb'][j][:],
                             xdbl[:])
        hs = [[None] * P, [None] * P]
        for mi in range(P):
            t_u = _tile(sb, [128, CH], F32, "u")
            nc.vector.tensor_mul(t_u[:], dt_sb[mi][:], xc_sp[mi][:, lcs])
            for n in range(2):
                t_dbu = _tile(sb, [128, CH], F32, f"dbu{n}")
                nc.vector.tensor_mul(t_dbu[:], t_u[:],
                                     repB[:, n * CH:(n + 1) * CH])
                t_hs = _tile(sb, [128, CH], F32, f"hs{n}", 3)
                init = 0.0 if c0 == 0 else hs_prev[n][mi][:, CH - 1:CH]
                nc.vector.tensor_tensor_scan(t_hs[:], dA_sb[n][mi][:],
                                             t_dbu[:], init, ALU.mult, ALU.add)
                hs[n][mi] = t_hs
                hs_prev[n][mi] = t_hs

        repC = _tile(ps_rep, [128, 2 * CH], F32, "repC", 1)
        for j in range(2):
            nc.tensor.matmul(repC[:, j * CH:(j + 1) * CH],
                             cfg['sel_sb'][2 + j][:], xdbl[:])

        # ---- y = (hs0*C0 + hs1*C1 + dd*xc) * silu(z) ----
        yg_sb = []
        for mi in range(P):
            t_m0 = _tile(sb, [128, CH], F32, "m0")
            nc.vector.tensor_mul(t_m0[:], hs[0][mi][:], repC[:, 0:CH])
            t_y = _tile(sb, [128, CH], F32, "y")
            nc.vector.tensor_mul(t_y[:], hs[1][mi][:], repC[:, CH:2 * CH])
            nc.vector.tensor_add(t_y[:], t_y[:], t_m0[:])
            nc.vector.scalar_tensor_tensor(t_y[:], xc_sp[mi][:, lcs],
                                           cols[mi][:, 5:6], t_y[:],
                                           ALU.mult, ALU.add)
            t_yg = _tile(sb, [128, CH], F32, "yg")
            nc.vector.tensor_mul(t_yg[:], t_y[:], sz_sp[mi][:, lcs])
            yg_sb.append(t_yg)

        # ---- out_proj (time-major) + LayerNorm ----
        nsub = CH // SUB
        ssum = _tile(sb, [SUB, nsub], F32, "ssum")
        ssq = _tile(sb, [SUB, nsub], F32, "ssq")
        yp_tiles = []
        for g in range(nsub):
            cs = slice(g * SUB, (g + 1) * SUB)
            yp_ps = _tile(ps_o, [SUB, dout], F32, "yp", 1)
            for mi in range(P):
                nc.tensor.matmul(yp_ps[:], yg_sb[mi][:, cs], wouts[mi][:],
                                 start=(mi == 0), stop=(mi == P - 1))
            yp = _tile(sb, [SUB, dout], F32, "ypsb", 4)
            nc.scalar.activation(yp[:], yp_ps[:], AF.Identity,
                                 accum_out=ssum[:, g:g + 1])
            scr = _tile(sb, [SUB, dout], F32, "scr")
            nc.scalar.activation(scr[:], yp_ps[:], AF.Square,
                                 accum_out=ssq[:, g:g + 1])
            yp_tiles.append(yp)
        mu = _tile(sb, [SUB, nsub], F32, "mu")
        nc.vector.tensor_scalar(mu[:], ssum[:], 1.0 / dout, None,
                                ALU.mult, ALU.bypass)
        musq = _tile(sb, [SUB, nsub], F32, "musq")
        nc.vector.tensor_mul(musq[:], mu[:], mu[:])
        var = _tile(sb, [SUB, nsub], F32, "var")
        nc.vector.scalar_tensor_tensor(var[:], ssq[:], 1.0 / dout, musq[:],
                                       ALU.mult, ALU.subtract)
        nc.vector.tensor_scalar(var[:], var[:], LN_EPS, None,
                                ALU.add, ALU.bypass)
        lnv = _tile(sb, [SUB, nsub], F32, "lnv")
        nc.scalar.activation(lnv[:], var[:], AF.Ln)
        rstd = _tile(sb, [SUB, nsub], F32, "rstd")
        nc.scalar.activation(rstd[:], lnv[:], AF.Exp, scale=-0.5)
        for g in range(nsub):
            tn = _tile(sb, [SUB, dout], cfg['tn_dtype'], "tn")
            nc.vector.tensor_scalar(tn[:], yp_tiles[g][:], mu[:, g:g + 1],
                                    rstd[:, g:g + 1], ALU.subtract, ALU.mult)
            cfg['emit'](tn, c0, g)


def build_program(L=4096, use_bf16=False):
    nc = bacc.Bacc()
    dp = nc.declare_dram_parameter
    x_d = dp("x", [128, L], F32, isOutput=False)
    w1k_d = dp("w1k", [4, 128, 128], F32, isOutput=False)
    w1z_d = dp("w1z", [128, 128], F32, isOutput=False)
    wx1_d = dp("wx1", [128, 12], F32, isOutput=False)
    wdt1_d = dp("wdt1", [8, 128], F32, isOutput=False)
    wout1_d = dp("wout1", [128, 128], F32, isOutput=False)
    cols1_d = dp("cols1", [128, 9], F32, isOutput=False)
    w2k_d = dp("w2k", [4, 256, 256], F32, isOutput=False)
    w2z_d = dp("w2z", [256, 256], F32, isOutput=False)
    wx2_d = dp("wx2", [256, 20], F32, isOutput=False)
    wdt2_d = dp("wdt2", [16, 256], F32, isOutput=False)
    wout2_d = dp("wout2", [256, 256], F32, isOutput=False)
    cols2_d = dp("cols2", [256, 11], F32, isOutput=False)
    sel1_d = dp("sel1", [4, 12, 128], F32, isOutput=False)
    sel2_d = dp("sel2", [4, 20, 128], F32, isOutput=False)
    linw_d = dp("linw", [128, 256], BF16 if use_bf16 else F32, isOutput=False)
    linb_d = dp("linb", [256, 1], F32, isOutput=False)
    out_d = dp("out", [256, L], F32, isOutput=True)

    dma = nc.sync.dma_start
    t1_dtype = BF16 if use_bf16 else F32

    with tile.TileContext(nc) as tc, ExitStack() as ctx:
        consts = ctx.enter_context(tc.tile_pool(name="consts", bufs=1))
        planes = ctx.enter_context(tc.tile_pool(name="planes", bufs=1))
        spans = ctx.enter_context(tc.tile_pool(name="spans", bufs=1))
        sb = ctx.enter_context(tc.tile_pool(name="sb", bufs=2))
        ps_mm = ctx.enter_context(
            tc.tile_pool(name="psmm", bufs=2, space=bass.MemorySpace.PSUM))
        ps_rep = ctx.enter_context(
            tc.tile_pool(name="psrep", bufs=1, space=bass.MemorySpace.PSUM))
        ps_o = ctx.enter_context(
            tc.tile_pool(name="pso", bufs=1, space=bass.MemorySpace.PSUM))
        pools = {'sb': sb, 'mm': ps_mm, 'rep': ps_rep, 'o': ps_o}

        _ld = [0]

        def load(dram_ap, shape, dtype=F32):
            _ld[0] += 1
            t = consts.tile(shape, dtype, tag=f"w{_ld[0]}", name=f"w{_ld[0]}")
            dma(t[:], dram_ap)
            return t

        w1k_sb = [[load(w1k_d[k], [128, 128])] for k in range(4)]
        w1z_sb = [load(w1z_d[:], [128, 128])]
        wx1_sb = [load(wx1_d[:], [128, 12])]
        wdt1_sb = load(wdt1_d[:], [8, 128])
        wout1_sb = [load(wout1_d[:], [128, 128])]
        cols1_sb = [load(cols1_d[:], [128, 9])]
        w2k_sb = [[load(w2k_d[k, kt * 128:(kt + 1) * 128], [128, 256])
                   for kt in range(2)] for k in range(4)]
        w2z_sb = [load(w2z_d[kt * 128:(kt + 1) * 128], [128, 256])
                  for kt in range(2)]
        wx2_sb = [load(wx2_d[kt * 128:(kt + 1) * 128], [128, 20])
                  for kt in range(2)]
        wdt2_sb = load(wdt2_d[:], [16, 256])
        wout2_sb = [load(wout2_d[kt * 128:(kt + 1) * 128], [128, 256])
                    for kt in range(2)]
        cols2_sb = [load(cols2_d[kt * 128:(kt + 1) * 128], [128, 11])
                    for kt in range(2)]
        sel1_sb = [load(sel1_d[j], [12, 128]) for j in range(4)]
        sel2_sb = [load(sel2_d[j], [20, 128]) for j in range(4)]
        linw_sb = load(linw_d[:], [128, 256], BF16 if use_bf16 else F32)
        linb_sb = [load(linb_d[kt * 128:(kt + 1) * 128], [128, 1])
                   for kt in range(2)]

        ident = consts.tile([128, 128], F32, tag="ident", name="ident")
        make_identity(nc, ident)

        xpad = planes.tile([128, L + 3], F32, tag="xpad", name="xpad")
        nc.gpsimd.memset(xpad[:, 0:3], 0.0)
        dma(xpad[:, 3:], x_d[:])
        t1n = planes.tile([128, L], t1_dtype, tag="t1n", name="t1n")
        t2pad = [planes.tile([128, L + 3], F32, tag=f"t2pad{mi}",
                             name=f"t2pad{mi}") for mi in range(2)]
        for mi in range(2):
            nc.gpsimd.memset(t2pad[mi][:, 0:3], 0.0)

        def span_tiles(P):
            xc_sp = [_tile(spans, [128, SPAN], F32, f"xcsp{mi}")
                     for mi in range(P)]
            sz_sp = [_tile(spans, [128, SPAN], F32, f"szsp{mi}")
                     for mi in range(P)]
            return xc_sp, sz_sp

        # ---- stage 1 ----
        if use_bf16:
            def emit1(tn, c0, g):
                nc.sync.dma_start_transpose(
                    t1n[:, c0 + g * SUB: c0 + (g + 1) * SUB], tn[:])
        else:
            def emit1(tn, c0, g):
                tf = _tile(ps_o, [128, SUB], F32, "tf", 1)
                nc.tensor.transpose(tf[:], tn[:], ident[:])
                nc.scalar.activation(
                    t1n[:, c0 + g * SUB: c0 + (g + 1) * SUB], tf[:],
                    AF.Identity)

        cfg1 = dict(
            L=L, P_in=1, P=1, r=8, dout=128, in_planes=[xpad],
            wk_sb=w1k_sb, wz_sb=w1z_sb, wx_sb=wx1_sb, wdt_sb=wdt1_sb,
            wout_sb=wout1_sb, cols_sb=cols1_sb, sel_sb=sel1_sb,
            tn_dtype=t1_dtype, emit=emit1)
        hs_prev1 = [[None], [None]]
        for s0 in range(0, L, SPAN):
            xc_sp, sz_sp = span_tiles(1)
            cfg1['xc_sp'], cfg1['sz_sp'] = xc_sp, sz_sp
            _stage_phase_a(nc, pools, cfg1, s0)
            _stage_phase_b(nc, pools, cfg1, s0, hs_prev1)

        # ---- stage 2 (the linear+SiLU joins each span's SiLU phase) ----
        def emit2(tn, c0, g):
            for ct in range(2):
                tf = _tile(ps_o, [128, SUB], F32, "tf", 1)
                nc.tensor.transpose(tf[:], tn[:, ct * 128:(ct + 1) * 128],
                                    ident[:])
                of = _tile(sb, [128, SUB], F32, "of")
                nc.vector.tensor_scalar(of[:], tf[:], cols2_sb[ct][:, 9:10],
                                        cols2_sb[ct][:, 10:11],
                                        ALU.mult, ALU.add)
                dma(out_d[ct * 128:(ct + 1) * 128,
                          c0 + g * SUB: c0 + (g + 1) * SUB], of[:])

        cfg2 = dict(
            L=L, P_in=2, P=2, r=16, dout=256, in_planes=t2pad,
            wk_sb=w2k_sb, wz_sb=w2z_sb, wx_sb=wx2_sb, wdt_sb=wdt2_sb,
            wout_sb=wout2_sb, cols_sb=cols2_sb, sel_sb=sel2_sb,
            tn_dtype=F32, emit=emit2)
        hs_prev2 = [[None, None], [None, None]]
        for s0 in range(0, L, SPAN):
            # linear + silu for this span (same SiLU table set as phase A)
            for c0 in range(s0, s0 + SPAN, CH):
                for mi in range(2):
                    ms = slice(mi * 128, (mi + 1) * 128)
                    lp = _tile(ps_mm, [128, CH], F32, "mm", 2)
                    nc.tensor.matmul(lp[:], linw_sb[:, ms], t1n[:, c0:c0 + CH])
                    nc.scalar.activation(t2pad[mi][:, 3 + c0: 3 + c0 + CH],
                                         lp[:], AF.Silu,
                                         bias=linb_sb[mi][:, 0:1])
            xc_sp, sz_sp = span_tiles(2)
            cfg2['xc_sp'], cfg2['sz_sp'] = xc_sp, sz_sp
            _stage_phase_a(nc, pools, cfg2, s0)
            _stage_phase_b(nc, pools, cfg2, s0, hs_prev2)

    nc.finalize()
    return nc


# ----------------------------------------------------------------------------
# entry point
# ----------------------------------------------------------------------------

_NC = {}


def kernel(**inputs):
    global last_exec_time_ns
    use_bf16 = os.environ.get("KBENCH_BF16", "0") == "1"
    inputs = {k: np.asarray(v) for k, v in inputs.items()}
    weights = prep_weights(inputs, use_bf16)
    x = inputs['x'].astype(np.float32)          # [8, 128, 64, 64]
    b, c, h, w = x.shape
    L = h * w

    key = (L, use_bf16)
    if key not in _NC:
        _NC[key] = build_program(L, use_bf16)

    in_maps = [dict(weights, x=np.ascontiguousarray(x[i].reshape(c, L)))
               for i in range(NCORES)]
    res = run_bass_kernel_spmd(
        _NC[key], in_maps, list(range(NCORES)),
        trace=bool(os.environ.get("KBENCH_TRACE")))
    last_exec_time_ns = res.exec_time_ns
    out = np.stack([np.asarray(res.results[i]['out'], np.float32)
                    .reshape(256, h, w) for i in range(NCORES)])
    return out
